# revision 8
# baseline (speedup 1.0000x reference)
"""DyConv (MoE-routed dynamic convolution) Trainium2 Bass kernel, v2.

Data-parallel over batch: 32 samples -> 8 cores x 4 samples.

Differences vs v1 (the 7763ns baseline):
  - bf16 end-to-end on the conv path: x (DMA'd as bf16), expert bank,
    mixed kernels, and output staging. bf16 matmuls run the PE at
    1 cycle/row (same as fp32r) but get FWL (fast weight load, 2x) and
    separate LDWEIGHTS instructions that the PE's 64-deep reorder window
    prefetches during the previous matmul -- so weight loads are hidden,
    where the fp32r path's fused S3_LW loads were serial.
  - x is zero-padded on the HOST to [BL, 128, 58, 58] and DMA'd directly
    into the padded SBUF image: the per-sample DVE pad-copy is gone and
    x DMA traffic is halved.
  - Output is staged to SBUF as bf16 (ACT PSUM->SBUF copy) and DMA'd as
    bf16 (host converts back to fp32): out traffic halved.
  - Rotated software pipeline with 2-sample lookahead: the router chain
    for sample j runs embedded inside conv(j-2)'s matmul stream, each
    router matmul placed several conv tiles after its producer so its
    deps are satisfied when PE reaches it. PE never idles more than the
    loop barrier, keeping the HAM clock gate at 2.4 GHz (the v1 kernel's
    per-sample router stalls kept re-throttling it to 1.2 GHz).
  - In-DMAs ride the SP HWDGE queue, out-DMAs the ACT queue.

Error: bf16 rounding of x and kern gives ~2e-3 max-rel error vs the
fp32 reference (threshold 2e-2).
"""

import os
from contextlib import ExitStack

import numpy as np

import concourse.bass as bass
import concourse.bacc as bacc
import concourse.tile as tile
from concourse import mybir
from concourse.bass_utils import run_bass_kernel_spmd

F32 = mybir.dt.float32
BF16 = mybir.dt.bfloat16

B, CIN, H, W = 32, 128, 56, 56
COUT, KS, E, R = 256, 3, 4, 16
NCORES = 8
BL = B // NCORES
TEMP = 30.0
HP, WP = H + 2, W + 2
HWN = H * W
ROWS = 8
NTILES = H // ROWS  # 7
NFREE = ROWS * W  # 448
HALFCO = 1152  # 9 taps * 128 cout per half
EBLK = 2 * HALFCO  # 2304 per expert

# taps kh-major; ki = kh*3+kw, weight col block ki*128 inside each half
TAPS = [(dh, dw) for dh in (-1, 0, 1) for dw in (-1, 0, 1)]

TRACE = os.environ.get("DYCONV_TRACE", "0") == "1"
LAST_RESULTS = None
LOOP_REPS = int(os.environ.get("DYCONV_LOOP_REPS", "1"))
# timing builds: big tensors Internal (zero-initialized on device), tiny
# dummy in / sink out so per-call host<->device traffic is negligible
TIMING = os.environ.get("DYCONV_TIMING", "0") == "1"
# full | noout (skip out DMAs) | peonly (also skip stage copies)
MODE = os.environ.get("DYCONV_MODE", "full")
# tap-outer conv order: blocks of 3 row tiles share each stationary weight
# (LDWEIGHTS count 126 -> 54 per sample); needs 6 conv PSUM banks.
TAPO = os.environ.get("DYCONV_TAPO", "0") == "1"
# emit N serial copies of the loop body instead of a hardware For_i loop
# (TimelineSim can't resolve reg-mode loop branches; unrolled bodies can be
# simulated and profiled per the cost model)
UNROLL = int(os.environ.get("DYCONV_UNROLL", "0"))
# timing bisect: prep all 4 samples in the prologue, loop pure conv bodies
# (no per-sample DMA/gap/router/mix in steady state -- isolates the conv
# stream + stage/out path from prep-engine contention)
NOPREP = os.environ.get("DYCONV_NOPREP", "0") == "1"
# which engine's HWDGE queue carries the out DMAs (act was v2's choice)
OUTQ = os.environ.get("DYCONV_OUTQ", "act")
# strip per-matmul EVT-semaphore increments nobody waits on (each inc is a
# serialized ~26ns EVT_SEM register write on PE), renumbering all waits.
STRIP = os.environ.get("DYCONV_STRIP", "1") == "1"
# delete InstLdweights whose weights AP is identical to the previous one
# (only useful with TAPO=1, where tap-blocks repeat the stationary weight)
LDWDEDUP = os.environ.get("DYCONV_LDWDEDUP", "0") == "1"
# contiguous conv rhs: flat 464-element windows over padded rows (guard
# element at each end keeps all 9 tap offsets in-bounds; junk columns are
# dropped at the stage-copy). Tests whether strided-row APs cost PE cycles.
CONTIG = os.environ.get("DYCONV_CONTIG", "0") == "1"
NFREE_C = ROWS * WP  # 464


def _build_program():
    nc = bacc.Bacc("TRN2", target_bir_lowering=False, debug=False)
    kind = "Internal" if TIMING else "ExternalInput"
    okind = "Internal" if TIMING else "ExternalOutput"
    x_d = nc.dram_tensor("x", [BL, CIN, HP, WP], BF16, kind=kind).ap()
    convs_d = nc.dram_tensor("convs", [CIN, E * EBLK], BF16, kind=kind).ap()
    w1t_d = nc.dram_tensor("w1t", [CIN, R], F32, kind=kind).ap()
    b1_d = nc.dram_tensor("b1", [R, 1], F32, kind=kind).ap()
    w2t_d = nc.dram_tensor("w2t", [R, E], F32, kind=kind).ap()
    g_d = nc.dram_tensor("g", [1, E], F32, kind=kind).ap()
    out_d = nc.dram_tensor("out", [BL, COUT, H, W], BF16, kind=okind).ap()
    if TIMING:
        dummy_d = nc.dram_tensor("din", [1, 1], F32, kind="ExternalInput").ap()
        sink_d = nc.dram_tensor("sink", [1, 1], F32, kind="ExternalOutput").ap()
    else:
        dummy_d = sink_d = None

    with tile.TileContext(nc) as tc, ExitStack() as ctx:
        _emit_all(ctx, tc, x_d, convs_d, w1t_d, b1_d, w2t_d, g_d, out_d,
                  dummy_d, sink_d)
    nc.compile()
    if STRIP:
        _strip_pe_sem_updates(nc)
    if LDWDEDUP:
        _dedup_ldweights(nc)
    return nc


def _emit_all(ctx, tc, x_d, convs_d, w1t_d, b1_d, w2t_d, g_d, out_d,
              dummy_d, sink_d):
    nc = tc.nc

    const_pool = ctx.enter_context(tc.tile_pool(name="const", bufs=1))
    xpr_pool = ctx.enter_context(tc.tile_pool(name="xpr", bufs=4))
    kern_pool = ctx.enter_context(tc.tile_pool(name="kern", bufs=4))
    small_pool = ctx.enter_context(tc.tile_pool(name="small", bufs=2))
    stage_pool = ctx.enter_context(tc.tile_pool(name="stage", bufs=4))
    psum_pool = ctx.enter_context(tc.tile_pool(
        name="psum", bufs=6 if TAPO else 4, space="PSUM"))
    psum_r_pool = ctx.enter_context(tc.tile_pool(name="psum_r", bufs=2, space="PSUM"))

    # second in-queue: ACT normally; in TIMING builds everything stays on
    # SP so the zero-init writes below are strictly ordered before reads.
    eng2 = nc.sync if TIMING else nc.scalar
    oeng = {"act": nc.scalar, "sp": nc.sync, "gp": nc.gpsimd,
            "vec": nc.vector}[OUTQ]

    # ---- resident constants -------------------------------------------
    convs_sb = const_pool.tile([CIN, E * EBLK], BF16)
    # bf16 broadcast operands: the rb matmul with fp32 ones would use the
    # fused S3_LW 4-byte weight load (~107ns serialized in the PE stream);
    # bf16 gets a separate, prefetched LDWEIGHTS instead.
    ones_sb = const_pool.tile([1, CIN], BF16)
    nc.vector.memset(ones_sb[:], 1.0)
    zf = const_pool.tile([CIN, R], F32)
    nc.vector.memset(zf[:], 0.0)

    if TIMING:
        # zero-fill the Internal input tensors once so the loop never sees
        # uninitialized DRAM (NaN-safe); done before any read of them.
        zb = const_pool.tile([CIN, HP * WP], BF16)
        nc.vector.memset(zb[:], 0.0)
        for b in range(BL):
            nc.sync.dma_start(x_d[b], zb[:])
        for e in range(E):
            nc.sync.dma_start(convs_d[:, e * EBLK : (e + 1) * EBLK],
                              zb[:, 0:EBLK])
        nc.sync.dma_start(w1t_d[:], zf[:])
        nc.sync.dma_start(b1_d[:], zf[0:R, 0:1])
        nc.sync.dma_start(w2t_d[:], zf[0:R, 0:E])
        nc.sync.dma_start(g_d[:], zf[0:1, 0:E])

    w1t_sb = const_pool.tile([CIN, R], F32)
    nc.sync.dma_start(w1t_sb[:], w1t_d[:])
    b1_sb = const_pool.tile([R, 1], F32)
    nc.sync.dma_start(b1_sb[:], b1_d[:])
    w2t_sb = const_pool.tile([R, E], F32)
    nc.sync.dma_start(w2t_sb[:], w2t_d[:])
    g_sb = const_pool.tile([1, E], F32)
    nc.sync.dma_start(g_sb[:], g_d[:])

    # HAM pre-warm: ~4.5us of dependency-free matmuls that run during the
    # prologue's x/weights DMA latency (PE would be idle anyway), so the
    # HAM clock gate reaches 2.4 GHz before the first real conv matmul.
    wtile = const_pool.tile([CIN, NFREE], BF16)
    nc.vector.memset(wtile[:], 0.25)
    for wi in range(12):
        wps = psum_pool.tile([128, ROWS, W], F32, tag="cps", name=f"wps{wi}")
        nc.tensor.matmul(wps[:], lhsT=wtile[:, 0:128], rhs=wtile[:],
                         start=True, stop=True)

    # warmup matmuls absorb the router-weight DMA waits on PE's clock so
    # the per-sample fp32 router matmuls (single-sync-wait S3_LW encoding)
    # only ever need their one data dependency.
    warm1t = psum_r_pool.tile([CIN, R], F32, tag="rps")
    nc.tensor.matmul(warm1t[0:R, 0:R], lhsT=w1t_sb[:, 0:R], rhs=w1t_sb[:, 0:R],
                     start=True, stop=True)
    warm2t = psum_r_pool.tile([CIN, R], F32, tag="rps")
    nc.tensor.matmul(warm2t[0:E, 0:E], lhsT=w2t_sb[:, 0:E], rhs=w2t_sb[:, 0:E],
                     start=True, stop=True)

    state = {}  # j -> list of per-sample tiles

    def p_dma(j, engine):
        if CONTIG:
            xpr = xpr_pool.tile([CIN, HP * WP + 2], BF16, tag="xpr")
            engine.dma_start(xpr[:, 1 : 1 + HP * WP], x_d[j])
        else:
            xpr = xpr_pool.tile([CIN, HP, WP], BF16, tag="xpr")
            engine.dma_start(xpr[:], x_d[j])
        kern = kern_pool.tile([CIN, EBLK], BF16, tag="kern")
        state[j] = [xpr, kern]

    def p_gap(j):
        gap = small_pool.tile([CIN, 1], F32, tag="gap")
        if CONTIG:
            nc.vector.reduce_sum(gap[:], state[j][0][:, 1 : 1 + HP * WP],
                                 axis=mybir.AxisListType.X)
        else:
            nc.vector.reduce_sum(gap[:], state[j][0][:],
                                 axis=mybir.AxisListType.XY)
        state[j].append(gap)  # [2]

    def p_ph(j):
        pht = psum_r_pool.tile([CIN, R], F32, tag="rps")
        ph = pht[0:R, 0:1]
        nc.tensor.matmul(ph, lhsT=w1t_sb[:], rhs=state[j][2][:],
                         start=True, stop=True)
        hmid = small_pool.tile([R, 1], F32, tag="hmid")
        state[j] += [ph, hmid]  # [3], [4]

    def p_relu(j):
        nc.scalar.activation(state[j][4][:], state[j][3],
                             mybir.ActivationFunctionType.Relu,
                             bias=b1_sb[:], scale=1.0)

    def p_pl(j):
        plt = psum_r_pool.tile([CIN, R], F32, tag="rps")
        pl = plt[0:1, 0:E]
        nc.tensor.matmul(pl, lhsT=state[j][4][:], rhs=w2t_sb[:],
                         start=True, stop=True)
        state[j].append(pl)  # [5]

    def p_soft(j):
        ex = small_pool.tile([1, E], F32, tag="ex")
        nc.scalar.activation(ex[:], state[j][5], mybir.ActivationFunctionType.Exp,
                             scale=1.0 / TEMP)
        exg = small_pool.tile([1, E], F32, tag="exg")
        nc.vector.tensor_mul(exg[:], ex[:], g_sb[:])
        ssum = small_pool.tile([1, 1], F32, tag="ssum")
        nc.vector.reduce_sum(ssum[:], exg[:], axis=mybir.AxisListType.X)
        rec = small_pool.tile([1, 1], F32, tag="rec")
        nc.vector.reciprocal(rec[:], ssum[:])
        rt = small_pool.tile([1, E], BF16, tag="rt")
        nc.vector.tensor_scalar_mul(rt[:], exg[:], rec[:])
        state[j].append(rt)  # [6]

    def p_rb(j):
        rbt = psum_r_pool.tile([CIN, R], F32, tag="rps")
        rb = rbt[:, 0:E]
        nc.tensor.matmul(rb, lhsT=ones_sb[:], rhs=state[j][6][:],
                         start=True, stop=True)
        rb_sb = small_pool.tile([CIN, E], F32, tag="rb_sb")
        nc.vector.tensor_copy(rb_sb[:], rb)
        state[j].append(rb_sb)  # [7]

    def p_mix(j, c0, c1):
        # kern[:, c0:c1] = sum_e rb[e] * convs[:, e*EBLK + c0:c1]
        kern, rb_sb = state[j][1], state[j][7]
        nc.vector.tensor_scalar_mul(kern[:, c0:c1],
                                    convs_sb[:, c0:c1], rb_sb[:, 0:1])
        for e in range(1, E):
            nc.vector.scalar_tensor_tensor(
                kern[:, c0:c1], convs_sb[:, e * EBLK + c0 : e * EBLK + c1],
                rb_sb[:, e : e + 1], kern[:, c0:c1],
                op0=mybir.AluOpType.mult, op1=mybir.AluOpType.add)

    def prep_serial(j, engine):
        # prologue-only latency trims: the x DMA is split across both
        # in-queues and reduced in two halves (GAP starts after the first
        # half lands), and the mixing is chunked tap-aligned so conv(0)'s
        # first matmuls start ~1.2us after rb instead of after full mixing.
        if CONTIG:
            p_dma(j, engine)
            p_gap(j)
        else:
            xpr = xpr_pool.tile([CIN, HP, WP], BF16, tag="xpr", name="xpr")
            nc.sync.dma_start(xpr[:, 0:29, :], x_d[j][:, 0:29, :])
            eng2.dma_start(xpr[:, 29:HP, :], x_d[j][:, 29:HP, :])
            kern = kern_pool.tile([CIN, EBLK], BF16, tag="kern", name="kern")
            state[j] = [xpr, kern]
        if j == 0:
            # expert bank chunks split across both in-queues, queued after
            # the first two images so mix(0) can start earliest.
            for e in range(E):
                (nc.sync if (TIMING or e < 2) else nc.scalar).dma_start(
                    convs_sb[:, e * EBLK : (e + 1) * EBLK],
                    convs_d[:, e * EBLK : (e + 1) * EBLK])
        if not CONTIG:
            g0 = small_pool.tile([CIN, 1], F32, tag="gp0")
            nc.vector.reduce_sum(g0[:], state[j][0][:, 0:29, :],
                                 axis=mybir.AxisListType.XY)
            g1 = small_pool.tile([CIN, 1], F32, tag="gp1")
            nc.vector.reduce_sum(g1[:], state[j][0][:, 29:HP, :],
                                 axis=mybir.AxisListType.XY)
            gap = small_pool.tile([CIN, 1], F32, tag="gap")
            nc.vector.tensor_add(gap[:], g0[:], g1[:])
            state[j].append(gap)  # [2]
        p_ph(j)
        p_relu(j)
        p_pl(j)
        p_soft(j)
        p_rb(j)
        p_mix(j, 0, 384)
        p_mix(j, 384, HALFCO)
        p_mix(j, HALFCO, EBLK)

    def conv_tapo(b, j):
        """Tap-outer conv: blocks of 3 row tiles reuse each stationary
        weight. Embedded prep(j) hooks fire at block boundaries (>=3 conv
        tiles between a hook's producer and its dependent PE matmul)."""
        xpr, kern = state[b][0], state[b][1]
        if j is not None:
            p_dma(j, nc.sync)
        hooks = {
            (0, 0): lambda: p_gap(j),
            (0, 1): lambda: (p_ph(j), p_relu(j)),
            (0, 2): lambda: p_pl(j),
            (1, 0): lambda: p_soft(j),
            (1, 1): lambda: p_rb(j),
            (1, 2): lambda: (p_mix(j, 0, HALFCO), p_mix(j, HALFCO, EBLK)),
        }
        for half in range(2):
            if MODE != "peonly":
                stage = stage_pool.tile([128, H, W], BF16, tag="stage")
            for bi, (t0, blen) in enumerate([(0, 3), (3, 3), (6, 1)]):
                pss = [psum_pool.tile([128, ROWS, W], F32, tag="cps",
                                      name=f"ps{i}")
                       for i in range(blen)]
                for ki, (dh, dw) in enumerate(TAPS):
                    lhsT = kern[:, half * HALFCO + ki * 128
                                : half * HALFCO + ki * 128 + 128]
                    for i in range(blen):
                        t = t0 + i
                        rhs = xpr[:, 1 + ROWS * t + dh : 1 + ROWS * t + dh + ROWS,
                                  1 + dw : 1 + dw + W]
                        nc.tensor.matmul(pss[i][:], lhsT=lhsT, rhs=rhs,
                                         start=(ki == 0), stop=(ki == 8))
                if MODE != "peonly":
                    for i in range(blen):
                        t = t0 + i
                        nc.scalar.copy(stage[:, ROWS * t : ROWS * (t + 1), :],
                                       pss[i][:])
                if j is not None:
                    hooks[(half, bi)]()
            if MODE == "full":
                oeng.dma_start(
                    out_d[b, half * 128 : half * 128 + 128], stage[:])

    def conv(b, j):
        """Emit conv of sample b; if j is not None, embed prep(j) work at
        fixed points of the 14-tile stream (deps are then always satisfied
        several tiles before PE reaches each embedded router matmul)."""
        if TAPO:
            return conv_tapo(b, j)
        xpr, kern = state[b][0], state[b][1]
        if j is not None:
            p_dma(j, nc.sync)
        for half in range(2):
            if MODE != "peonly":
                stage = stage_pool.tile([128, H, W], BF16, tag="stage")
            for t in range(NTILES):
                g = half * NTILES + t  # 0..13
                if CONTIG:
                    ps = psum_pool.tile([128, ROWS, WP], F32, tag="cps")
                else:
                    ps = psum_pool.tile([128, ROWS, W], F32, tag="cps")
                for ki, (dh, dw) in enumerate(TAPS):
                    lhsT = kern[:, half * HALFCO + ki * 128
                                : half * HALFCO + ki * 128 + 128]
                    if CONTIG:
                        off = 1 + (1 + ROWS * t + dh) * WP + dw
                        rhs = xpr[:, off : off + NFREE_C]
                    else:
                        rhs = xpr[:, 1 + ROWS * t + dh
                                  : 1 + ROWS * t + dh + ROWS,
                                  1 + dw : 1 + dw + W]
                    nc.tensor.matmul(ps[:], lhsT=lhsT, rhs=rhs,
                                     start=(ki == 0), stop=(ki == 8))
                if MODE != "peonly":
                    src = ps[:, :, 1 : 1 + W] if CONTIG else ps[:]
                    nc.scalar.copy(stage[:, ROWS * t : ROWS * (t + 1), :], src)
                    # drain trim: the kernel's very last output half DMAs in
                    # two pieces so the first 32 rows fly while tiles 4-6
                    # still compute
                    if MODE == "full" and b == BL - 1 and half == 1 and t == 3:
                        oeng.dma_start(
                            out_d[b, 128:256][:, 0:32, :], stage[:, 0:32, :])
                if j is not None:
                    if g == 2:
                        p_gap(j)
                    elif g == 4:
                        p_ph(j)
                    elif g == 5:
                        p_relu(j)
                    elif g == 6:
                        p_pl(j)
                    elif g == 7:
                        p_soft(j)
                    elif g == 8:
                        p_rb(j)
                    elif g == 9:
                        p_mix(j, 0, HALFCO)
                    elif g == 10:
                        p_mix(j, HALFCO, EBLK)
            if MODE == "full":
                if b == BL - 1 and half == 1:
                    oeng.dma_start(
                        out_d[b, 128:256][:, 32:H, :], stage[:, 32:H, :])
                else:
                    oeng.dma_start(
                        out_d[b, half * 128 : half * 128 + 128], stage[:])

    def body(include_next):
        for b in range(BL):
            j = (b + 2) % 4
            if not include_next and b >= 2:
                j = None
            if NOPREP:
                j = None
            conv(b, j)

    # prologue: router+kern for samples 0 and 1 (x0 in-DMA on SP, x1 on the
    # ACT queue so the two transfers overlap)
    prep_serial(0, nc.sync)
    prep_serial(1, eng2)
    if NOPREP:
        p_dma(2, nc.sync)
        p_gap(2); p_ph(2); p_relu(2); p_pl(2); p_soft(2); p_rb(2)
        p_mix(2, 0, EBLK)
        p_dma(3, eng2)
        p_gap(3); p_ph(3); p_relu(3); p_pl(3); p_soft(3); p_rb(3)
        p_mix(3, 0, EBLK)

    if UNROLL > 0:
        for _ in range(UNROLL):
            body(include_next=True)
    elif LOOP_REPS > 1:
        with tc.For_i(0, LOOP_REPS, 1, hint_engines=(mybir.EngineType.PE,)):
            body(include_next=True)
    else:
        body(include_next=False)

    if MODE != "full" and not TIMING:
        dummy_stage = stage_pool.tile([128, H, W], BF16, tag="stage")
        nc.vector.memset(dummy_stage[:], 0.0)
        nc.sync.dma_start(out_d[0, 0:128], dummy_stage[:])
    if TIMING:
        nc.sync.dma_start(sink_d[:], zf[0:1, 0:1])


def _dedup_ldweights(nc):
    """Delete an InstLdweights when the immediately-preceding surviving
    InstLdweights in the same block loaded the identical weights (same AP,
    perf mode, tile position) and the candidate carries no sync waits --
    the PE weight buffer still holds those weights, so the paired matmuls
    keep working and skip the reload."""
    fn = nc.m.functions[0]
    removed = 0
    for blk in fn.blocks:
        last_sig = None
        keep = []
        for i in blk.instructions:
            tn = type(i).__name__
            if tn == "InstLdweights":
                si = i.sync_info
                has_sync = si is not None and (len(si.on_wait) > 0
                                               or len(si.on_update) > 0)
                sig = (str(i.ins), str(i.perf_mode), str(i.tile_position),
                       str(i.is_transpose))
                if sig == last_sig and not has_sync:
                    removed += 1
                    continue  # drop it
                last_sig = sig
            elif tn != "InstMatmult":
                # any other PE-stream-relevant instruction invalidates
                # nothing (weights regs untouched by non-PE instructions),
                # but be conservative across control flow and barriers
                if tn in ("InstDrain", "InstEventSemaphore"):
                    pass  # sem ops don't touch the weight buffer
                else:
                    last_sig = None
            keep.append(i)
        if removed:
            blk.instructions.clear()
            for i in keep:
                blk.instructions.append(i)
    if removed:
        print(f"[kernel2] deduped {removed} InstLdweights")


def _strip_pe_sem_updates(nc):
    """Remove sem-inc updates from matmuls whose completion count nobody
    waits on, renumbering every wait/add/sub on that semaphore.

    The tile framework gives every PE instruction a then-inc on the PE
    event semaphore; consumers wait on absolute counts. Only ~14% of the
    counts are ever waited on (the last matmul of each PSUM tile and the
    router matmuls), but each inc costs a serialized EVT_SEM register
    write (~26ns) on the engine -- ~13us/iteration for 500+ matmuls.

    Model: a virtual count axis in block-layout order (which matches how
    the framework numbers waits: prologue counts, then body counts after
    the loop's sub-imm reset returns the value to the prologue total).
    """
    fn = nc.m.functions[0]
    sids = set()
    for blk in fn.blocks:
        for i in blk.instructions:
            if type(i).__name__ == "InstMatmult" and i.sync_info:
                for u in i.sync_info.on_update:
                    sids.add(u.id)
    if len(sids) != 1:
        return
    sid = sids.pop()

    mm_updates = []  # (inst, cum) for sem-inc-1 updates on sid
    waits = []       # SyncWait objects on sid
    addsubs = []     # (update_obj, cum_at_layout_pos)
    cum = 0
    ok = True
    for blk in fn.blocks:
        for i in blk.instructions:
            si = i.sync_info
            if si is None:
                continue
            for w in si.on_wait:
                if w.id == sid:
                    if w.wait_mode != "sem-ge-imm":
                        ok = False
                    waits.append(w)
            for u in si.on_update:
                if u.id != sid:
                    continue
                if u.update_mode == "sem-inc" and u.update_value == 1 \
                        and type(i).__name__ == "InstMatmult":
                    cum += 1
                    mm_updates.append((i, cum))
                elif u.update_mode in ("sem-add-imm", "sem-sub-imm"):
                    addsubs.append((u, cum))
                else:
                    ok = False
    if not ok or not mm_updates:
        return

    kept = sorted({v for w in waits for v in [w.wait_value]}
                  | {cum}
                  | {c + u.update_value for u, c in addsubs}
                  | {c for u, c in addsubs})
    import bisect

    def newv(v):
        return bisect.bisect_right(kept, v)

    kept_set = set(kept)
    removed = 0
    for inst, c in mm_updates:
        if c not in kept_set:
            si = inst.sync_info
            si.on_update = [u for u in si.on_update if u.id != sid]
            removed += 1
    for w in waits:
        w.wait_value = newv(w.wait_value)
    for u, c in addsubs:
        u.update_value = newv(c + u.update_value) - newv(c)
    log_note = f"stripped {removed}/{len(mm_updates)} PE sem updates"
    print(f"[kernel2] {log_note}")


_PROGRAM = None


def make_in_maps(inputs):
    import ml_dtypes

    x = np.asarray(inputs["x"], dtype=np.float32)
    convs = np.asarray(inputs["convs"], dtype=np.float32)
    w1 = np.asarray(inputs["w1"], dtype=np.float32)
    b1 = np.asarray(inputs["b1"], dtype=np.float32)
    w2 = np.asarray(inputs["w2"], dtype=np.float32)
    b2 = np.asarray(inputs["b2"], dtype=np.float32)

    xp = np.pad(x, ((0, 0), (0, 0), (1, 1), (1, 1))).astype(ml_dtypes.bfloat16)
    a = convs.transpose(2, 0, 1, 3, 4)          # [ci, e, co, kh, kw]
    a = a.reshape(CIN, E, 2, 128, KS, KS)       # co -> (half, co128)
    a = a.transpose(0, 1, 2, 4, 5, 3)           # [ci, e, half, kh, kw, co128]
    convs_r = np.ascontiguousarray(a).reshape(CIN, E * EBLK).astype(
        ml_dtypes.bfloat16)
    w1t = np.ascontiguousarray(w1.T) / float(HWN)
    b1c = np.ascontiguousarray(b1[:, None])
    w2t = np.ascontiguousarray(w2.T)
    g = np.ascontiguousarray(np.exp(b2 / TEMP)[None, :]).astype(np.float32)

    if TIMING:
        return [{"din": np.zeros((1, 1), np.float32)} for _ in range(NCORES)]
    return [
        {
            "x": np.ascontiguousarray(xp[c * BL : (c + 1) * BL]),
            "convs": convs_r,
            "w1t": w1t,
            "b1": b1c,
            "w2t": w2t,
            "g": g,
        }
        for c in range(NCORES)
    ]


def kernel(x, convs, w1, b1, w2, b2):
    global _PROGRAM, LAST_RESULTS
    if _PROGRAM is None:
        _PROGRAM = _build_program()
    nc = _PROGRAM

    in_maps = make_in_maps(dict(x=x, convs=convs, w1=w1, b1=b1, w2=w2, b2=b2))
    res = run_bass_kernel_spmd(nc, in_maps, core_ids=list(range(NCORES)),
                               trace=TRACE)
    LAST_RESULTS = res
    out = np.concatenate(
        [res.results[c]["out"].astype(np.float32) for c in range(NCORES)],
        axis=0)
    return out



# revision 11
# speedup vs baseline: 1.2596x; 1.2596x over previous
"""DyConv (MoE-routed dynamic convolution) Trainium2 Bass kernel, v2.

Data-parallel over batch: 32 samples -> 8 cores x 4 samples.

Differences vs v1 (the 7763ns baseline):
  - bf16 end-to-end on the conv path: x (DMA'd as bf16), expert bank,
    mixed kernels, and output staging. bf16 matmuls run the PE at
    1 cycle/row (same as fp32r) but get FWL (fast weight load, 2x) and
    separate LDWEIGHTS instructions that the PE's 64-deep reorder window
    prefetches during the previous matmul -- so weight loads are hidden,
    where the fp32r path's fused S3_LW loads were serial.
  - x is zero-padded on the HOST to [BL, 128, 58, 58] and DMA'd directly
    into the padded SBUF image: the per-sample DVE pad-copy is gone and
    x DMA traffic is halved.
  - Output is staged to SBUF as bf16 (ACT PSUM->SBUF copy) and DMA'd as
    bf16 (host converts back to fp32): out traffic halved.
  - Rotated software pipeline with 2-sample lookahead: the router chain
    for sample j runs embedded inside conv(j-2)'s matmul stream, each
    router matmul placed several conv tiles after its producer so its
    deps are satisfied when PE reaches it. PE never idles more than the
    loop barrier, keeping the HAM clock gate at 2.4 GHz (the v1 kernel's
    per-sample router stalls kept re-throttling it to 1.2 GHz).
  - In-DMAs ride the SP HWDGE queue, out-DMAs the ACT queue.

Error: bf16 rounding of x and kern gives ~2e-3 max-rel error vs the
fp32 reference (threshold 2e-2).
"""

import os
from contextlib import ExitStack

import numpy as np

import concourse.bass as bass
import concourse.bacc as bacc
import concourse.tile as tile
from concourse import mybir
from concourse.bass_utils import run_bass_kernel_spmd

F32 = mybir.dt.float32
BF16 = mybir.dt.bfloat16

B, CIN, H, W = 32, 128, 56, 56
COUT, KS, E, R = 256, 3, 4, 16
NCORES = 8
BL = B // NCORES
TEMP = 30.0
HP, WP = H + 2, W + 2
HWN = H * W
ROWS = 8
NTILES = H // ROWS  # 7
NFREE = ROWS * W  # 448
HALFCO = 1152  # 9 taps * 128 cout per half
EBLK = 2 * HALFCO  # 2304 per expert

# taps kh-major; ki = kh*3+kw, weight col block ki*128 inside each half
TAPS = [(dh, dw) for dh in (-1, 0, 1) for dw in (-1, 0, 1)]

TRACE = os.environ.get("DYCONV_TRACE", "0") == "1"
LAST_RESULTS = None
LOOP_REPS = int(os.environ.get("DYCONV_LOOP_REPS", "1"))
# timing builds: big tensors Internal (zero-initialized on device), tiny
# dummy in / sink out so per-call host<->device traffic is negligible
TIMING = os.environ.get("DYCONV_TIMING", "0") == "1"
# full | noout (skip out DMAs) | peonly (also skip stage copies)
# | constout (stage copies as noout; out DMAs read a constant buffer with
#   no stage dependency -- isolates DMA byte traffic from dep chains)
MODE = os.environ.get("DYCONV_MODE", "full")
# tap-outer conv order: blocks of 3 row tiles share each stationary weight
# (LDWEIGHTS count 126 -> 54 per sample); needs 6 conv PSUM banks.
TAPO = os.environ.get("DYCONV_TAPO", "0") == "1"
# emit N serial copies of the loop body instead of a hardware For_i loop
# (TimelineSim can't resolve reg-mode loop branches; unrolled bodies can be
# simulated and profiled per the cost model)
UNROLL = int(os.environ.get("DYCONV_UNROLL", "0"))
# timing bisect: prep all 4 samples in the prologue, loop pure conv bodies
# (no per-sample DMA/gap/router/mix in steady state -- isolates the conv
# stream + stage/out path from prep-engine contention)
NOPREP = os.environ.get("DYCONV_NOPREP", "0") == "1"
# which engine's HWDGE queue carries the out DMAs (act was v2's choice)
OUTQ = os.environ.get("DYCONV_OUTQ", "act")
# strip per-matmul EVT-semaphore increments nobody waits on (each inc is a
# serialized ~26ns EVT_SEM register write on PE), renumbering all waits.
STRIP = os.environ.get("DYCONV_STRIP", "1") == "1"
# delete InstLdweights whose weights AP is identical to the previous one
# (only useful with TAPO=1, where tap-blocks repeat the stationary weight)
LDWDEDUP = os.environ.get("DYCONV_LDWDEDUP", "0") == "1"
# contiguous conv rhs: flat 464-element windows over padded rows (guard
# element at each end keeps all 9 tap offsets in-bounds; junk columns are
# dropped at the stage-copy). Tests whether strided-row APs cost PE cycles.
CONTIG = os.environ.get("DYCONV_CONTIG", "0") == "1"
NFREE_C = ROWS * WP  # 464


def _build_program():
    nc = bacc.Bacc("TRN2", target_bir_lowering=False, debug=False)
    kind = "Internal" if TIMING else "ExternalInput"
    okind = "Internal" if TIMING else "ExternalOutput"
    x_d = nc.dram_tensor("x", [BL, CIN, HP, WP], BF16, kind=kind).ap()
    convs_d = nc.dram_tensor("convs", [CIN, E * EBLK], BF16, kind=kind).ap()
    w1t_d = nc.dram_tensor("w1t", [CIN, R], F32, kind=kind).ap()
    b1_d = nc.dram_tensor("b1", [R, 1], F32, kind=kind).ap()
    w2t_d = nc.dram_tensor("w2t", [R, E], F32, kind=kind).ap()
    g_d = nc.dram_tensor("g", [1, E], F32, kind=kind).ap()
    out_d = nc.dram_tensor("out", [BL, COUT, H, W], BF16, kind=okind).ap()
    if TIMING:
        dummy_d = nc.dram_tensor("din", [1, 1], F32, kind="ExternalInput").ap()
        sink_d = nc.dram_tensor("sink", [1, 1], F32, kind="ExternalOutput").ap()
    else:
        dummy_d = sink_d = None

    with tile.TileContext(nc) as tc, ExitStack() as ctx:
        _emit_all(ctx, tc, x_d, convs_d, w1t_d, b1_d, w2t_d, g_d, out_d,
                  dummy_d, sink_d)
    nc.compile()
    if STRIP:
        _strip_pe_sem_updates(nc)
    if LDWDEDUP:
        _dedup_ldweights(nc)
    return nc


def _emit_all(ctx, tc, x_d, convs_d, w1t_d, b1_d, w2t_d, g_d, out_d,
              dummy_d, sink_d):
    nc = tc.nc

    const_pool = ctx.enter_context(tc.tile_pool(name="const", bufs=1))
    xpr_pool = ctx.enter_context(tc.tile_pool(name="xpr", bufs=4))
    kern_pool = ctx.enter_context(tc.tile_pool(name="kern", bufs=4))
    small_pool = ctx.enter_context(tc.tile_pool(name="small", bufs=2))
    stage_pool = ctx.enter_context(tc.tile_pool(name="stage", bufs=4))
    psum_pool = ctx.enter_context(tc.tile_pool(
        name="psum", bufs=6 if TAPO else 4, space="PSUM"))
    psum_r_pool = ctx.enter_context(tc.tile_pool(name="psum_r", bufs=2, space="PSUM"))

    # second in-queue: ACT normally; in TIMING builds everything stays on
    # SP so the zero-init writes below are strictly ordered before reads.
    eng2 = nc.sync if TIMING else nc.scalar
    oeng = {"act": nc.scalar, "sp": nc.sync, "gp": nc.gpsimd,
            "vec": nc.vector}[OUTQ]

    # ---- resident constants -------------------------------------------
    convs_sb = const_pool.tile([CIN, E * EBLK], BF16)
    # bf16 broadcast operands: the rb matmul with fp32 ones would use the
    # fused S3_LW 4-byte weight load (~107ns serialized in the PE stream);
    # bf16 gets a separate, prefetched LDWEIGHTS instead.
    ones_sb = const_pool.tile([1, CIN], BF16)
    nc.vector.memset(ones_sb[:], 1.0)
    zf = const_pool.tile([CIN, R], F32)
    nc.vector.memset(zf[:], 0.0)

    if TIMING:
        # zero-fill the Internal input tensors once so the loop never sees
        # uninitialized DRAM (NaN-safe); done before any read of them.
        zb = const_pool.tile([CIN, HP * WP], BF16)
        nc.vector.memset(zb[:], 0.0)
        for b in range(BL):
            nc.sync.dma_start(x_d[b], zb[:])
        for e in range(E):
            nc.sync.dma_start(convs_d[:, e * EBLK : (e + 1) * EBLK],
                              zb[:, 0:EBLK])
        nc.sync.dma_start(w1t_d[:], zf[:])
        nc.sync.dma_start(b1_d[:], zf[0:R, 0:1])
        nc.sync.dma_start(w2t_d[:], zf[0:R, 0:E])
        nc.sync.dma_start(g_d[:], zf[0:1, 0:E])

    w1t_sb = const_pool.tile([CIN, R], F32)
    nc.sync.dma_start(w1t_sb[:], w1t_d[:])
    b1_sb = const_pool.tile([R, 1], F32)
    nc.sync.dma_start(b1_sb[:], b1_d[:])
    w2t_sb = const_pool.tile([R, E], F32)
    nc.sync.dma_start(w2t_sb[:], w2t_d[:])
    g_sb = const_pool.tile([1, E], F32)
    nc.sync.dma_start(g_sb[:], g_d[:])

    # HAM pre-warm: ~4.5us of dependency-free matmuls that run during the
    # prologue's x/weights DMA latency (PE would be idle anyway), so the
    # HAM clock gate reaches 2.4 GHz before the first real conv matmul.
    wtile = const_pool.tile([CIN, NFREE], BF16)
    nc.vector.memset(wtile[:], 0.25)
    if MODE == "constout":
        conststage = const_pool.tile([128, H, W], BF16)
        nc.vector.memset(conststage[:], 0.125)
    for wi in range(12):
        wps = psum_pool.tile([128, ROWS, W], F32, tag="cps", name=f"wps{wi}")
        nc.tensor.matmul(wps[:], lhsT=wtile[:, 0:128], rhs=wtile[:],
                         start=True, stop=True)

    # warmup matmuls absorb the router-weight DMA waits on PE's clock so
    # the per-sample fp32 router matmuls (single-sync-wait S3_LW encoding)
    # only ever need their one data dependency.
    warm1t = psum_r_pool.tile([CIN, R], F32, tag="rps")
    nc.tensor.matmul(warm1t[0:R, 0:R], lhsT=w1t_sb[:, 0:R], rhs=w1t_sb[:, 0:R],
                     start=True, stop=True)
    warm2t = psum_r_pool.tile([CIN, R], F32, tag="rps")
    nc.tensor.matmul(warm2t[0:E, 0:E], lhsT=w2t_sb[:, 0:E], rhs=w2t_sb[:, 0:E],
                     start=True, stop=True)

    state = {}  # j -> list of per-sample tiles

    def p_dma(j, engine):
        if CONTIG:
            xpr = xpr_pool.tile([CIN, HP * WP + 2], BF16, tag="xpr")
            engine.dma_start(xpr[:, 1 : 1 + HP * WP], x_d[j])
        else:
            xpr = xpr_pool.tile([CIN, HP, WP], BF16, tag="xpr")
            engine.dma_start(xpr[:], x_d[j])
        kern = kern_pool.tile([CIN, EBLK], BF16, tag="kern")
        state[j] = [xpr, kern]

    def p_gap(j):
        gap = small_pool.tile([CIN, 1], F32, tag="gap")
        if CONTIG:
            nc.vector.reduce_sum(gap[:], state[j][0][:, 1 : 1 + HP * WP],
                                 axis=mybir.AxisListType.X)
        else:
            nc.vector.reduce_sum(gap[:], state[j][0][:],
                                 axis=mybir.AxisListType.XY)
        state[j].append(gap)  # [2]

    def p_ph(j):
        pht = psum_r_pool.tile([CIN, R], F32, tag="rps")
        ph = pht[0:R, 0:1]
        nc.tensor.matmul(ph, lhsT=w1t_sb[:], rhs=state[j][2][:],
                         start=True, stop=True)
        hmid = small_pool.tile([R, 1], F32, tag="hmid")
        state[j] += [ph, hmid]  # [3], [4]

    def p_relu(j):
        nc.scalar.activation(state[j][4][:], state[j][3],
                             mybir.ActivationFunctionType.Relu,
                             bias=b1_sb[:], scale=1.0)

    def p_pl(j):
        plt = psum_r_pool.tile([CIN, R], F32, tag="rps")
        pl = plt[0:1, 0:E]
        nc.tensor.matmul(pl, lhsT=state[j][4][:], rhs=w2t_sb[:],
                         start=True, stop=True)
        state[j].append(pl)  # [5]

    def p_soft(j):
        ex = small_pool.tile([1, E], F32, tag="ex")
        nc.scalar.activation(ex[:], state[j][5], mybir.ActivationFunctionType.Exp,
                             scale=1.0 / TEMP)
        exg = small_pool.tile([1, E], F32, tag="exg")
        nc.vector.tensor_mul(exg[:], ex[:], g_sb[:])
        ssum = small_pool.tile([1, 1], F32, tag="ssum")
        nc.vector.reduce_sum(ssum[:], exg[:], axis=mybir.AxisListType.X)
        rec = small_pool.tile([1, 1], F32, tag="rec")
        nc.vector.reciprocal(rec[:], ssum[:])
        rt = small_pool.tile([1, E], BF16, tag="rt")
        nc.vector.tensor_scalar_mul(rt[:], exg[:], rec[:])
        state[j].append(rt)  # [6]

    def p_rb(j):
        rbt = psum_r_pool.tile([CIN, R], F32, tag="rps")
        rb = rbt[:, 0:E]
        nc.tensor.matmul(rb, lhsT=ones_sb[:], rhs=state[j][6][:],
                         start=True, stop=True)
        rb_sb = small_pool.tile([CIN, E], F32, tag="rb_sb")
        nc.vector.tensor_copy(rb_sb[:], rb)
        state[j].append(rb_sb)  # [7]

    def p_mix(j, c0, c1):
        # kern[:, c0:c1] = sum_e rb[e] * convs[:, e*EBLK + c0:c1]
        kern, rb_sb = state[j][1], state[j][7]
        nc.vector.tensor_scalar_mul(kern[:, c0:c1],
                                    convs_sb[:, c0:c1], rb_sb[:, 0:1])
        for e in range(1, E):
            nc.vector.scalar_tensor_tensor(
                kern[:, c0:c1], convs_sb[:, e * EBLK + c0 : e * EBLK + c1],
                rb_sb[:, e : e + 1], kern[:, c0:c1],
                op0=mybir.AluOpType.mult, op1=mybir.AluOpType.add)

    def prep_serial(j, engine):
        # prologue-only latency trims: the x DMA is split across both
        # in-queues and reduced in two halves (GAP starts after the first
        # half lands), and the mixing is chunked tap-aligned so conv(0)'s
        # first matmuls start ~1.2us after rb instead of after full mixing.
        if CONTIG:
            p_dma(j, engine)
            p_gap(j)
        else:
            xpr = xpr_pool.tile([CIN, HP, WP], BF16, tag="xpr", name="xpr")
            nc.sync.dma_start(xpr[:, 0:29, :], x_d[j][:, 0:29, :])
            eng2.dma_start(xpr[:, 29:HP, :], x_d[j][:, 29:HP, :])
            kern = kern_pool.tile([CIN, EBLK], BF16, tag="kern", name="kern")
            state[j] = [xpr, kern]
        if j == 0:
            # expert bank chunks split across both in-queues, queued after
            # the first two images so mix(0) can start earliest.
            for e in range(E):
                (nc.sync if (TIMING or e < 2) else nc.scalar).dma_start(
                    convs_sb[:, e * EBLK : (e + 1) * EBLK],
                    convs_d[:, e * EBLK : (e + 1) * EBLK])
        if not CONTIG:
            g0 = small_pool.tile([CIN, 1], F32, tag="gp0")
            nc.vector.reduce_sum(g0[:], state[j][0][:, 0:29, :],
                                 axis=mybir.AxisListType.XY)
            g1 = small_pool.tile([CIN, 1], F32, tag="gp1")
            nc.vector.reduce_sum(g1[:], state[j][0][:, 29:HP, :],
                                 axis=mybir.AxisListType.XY)
            gap = small_pool.tile([CIN, 1], F32, tag="gap")
            nc.vector.tensor_add(gap[:], g0[:], g1[:])
            state[j].append(gap)  # [2]
        p_ph(j)
        p_relu(j)
        p_pl(j)
        p_soft(j)
        p_rb(j)
        p_mix(j, 0, 384)
        p_mix(j, 384, HALFCO)
        p_mix(j, HALFCO, EBLK)

    def conv_tapo(b, j):
        """Tap-outer conv: blocks of 3 row tiles reuse each stationary
        weight. Embedded prep(j) hooks fire at block boundaries (>=3 conv
        tiles between a hook's producer and its dependent PE matmul)."""
        xpr, kern = state[b][0], state[b][1]
        if j is not None:
            p_dma(j, nc.sync)
        hooks = {
            (0, 0): lambda: p_gap(j),
            (0, 1): lambda: (p_ph(j), p_relu(j)),
            (0, 2): lambda: p_pl(j),
            (1, 0): lambda: p_soft(j),
            (1, 1): lambda: p_rb(j),
            (1, 2): lambda: (p_mix(j, 0, HALFCO), p_mix(j, HALFCO, EBLK)),
        }
        for half in range(2):
            if MODE != "peonly":
                stage = stage_pool.tile([128, H, W], BF16, tag="stage")
            for bi, (t0, blen) in enumerate([(0, 3), (3, 3), (6, 1)]):
                pss = [psum_pool.tile([128, ROWS, W], F32, tag="cps",
                                      name=f"ps{i}")
                       for i in range(blen)]
                for ki, (dh, dw) in enumerate(TAPS):
                    lhsT = kern[:, half * HALFCO + ki * 128
                                : half * HALFCO + ki * 128 + 128]
                    for i in range(blen):
                        t = t0 + i
                        rhs = xpr[:, 1 + ROWS * t + dh : 1 + ROWS * t + dh + ROWS,
                                  1 + dw : 1 + dw + W]
                        nc.tensor.matmul(pss[i][:], lhsT=lhsT, rhs=rhs,
                                         start=(ki == 0), stop=(ki == 8))
                if MODE != "peonly":
                    for i in range(blen):
                        t = t0 + i
                        nc.scalar.copy(stage[:, ROWS * t : ROWS * (t + 1), :],
                                       pss[i][:])
                if j is not None:
                    hooks[(half, bi)]()
            if MODE == "full":
                oeng.dma_start(
                    out_d[b, half * 128 : half * 128 + 128], stage[:])

    def conv(b, j):
        """Emit conv of sample b; if j is not None, embed prep(j) work at
        fixed points of the 14-tile stream (deps are then always satisfied
        several tiles before PE reaches each embedded router matmul)."""
        if TAPO:
            return conv_tapo(b, j)
        xpr, kern = state[b][0], state[b][1]
        if j is not None:
            p_dma(j, nc.sync)
        for half in range(2):
            if MODE != "peonly":
                stage = stage_pool.tile([128, H, W], BF16, tag="stage")
            for t in range(NTILES):
                g = half * NTILES + t  # 0..13
                if CONTIG:
                    ps = psum_pool.tile([128, ROWS, WP], F32, tag="cps")
                else:
                    ps = psum_pool.tile([128, ROWS, W], F32, tag="cps")
                for ki, (dh, dw) in enumerate(TAPS):
                    lhsT = kern[:, half * HALFCO + ki * 128
                                : half * HALFCO + ki * 128 + 128]
                    if CONTIG:
                        off = 1 + (1 + ROWS * t + dh) * WP + dw
                        rhs = xpr[:, off : off + NFREE_C]
                    else:
                        rhs = xpr[:, 1 + ROWS * t + dh
                                  : 1 + ROWS * t + dh + ROWS,
                                  1 + dw : 1 + dw + W]
                    nc.tensor.matmul(ps[:], lhsT=lhsT, rhs=rhs,
                                     start=(ki == 0), stop=(ki == 8))
                if MODE != "peonly":
                    src = ps[:, :, 1 : 1 + W] if CONTIG else ps[:]
                    nc.scalar.copy(stage[:, ROWS * t : ROWS * (t + 1), :], src)
                    # drain trim: the kernel's very last output half DMAs in
                    # two pieces so the first 32 rows fly while tiles 4-6
                    # still compute
                    if MODE == "full" and b == BL - 1 and half == 1 and t == 3:
                        oeng.dma_start(
                            out_d[b, 128:256][:, 0:32, :], stage[:, 0:32, :])
                if j is not None:
                    if g == 2:
                        p_gap(j)
                    elif g == 4:
                        p_ph(j)
                    elif g == 5:
                        p_relu(j)
                    elif g == 6:
                        p_pl(j)
                    elif g == 7:
                        p_soft(j)
                    elif g == 8:
                        p_rb(j)
                    elif g == 9:
                        p_mix(j, 0, HALFCO)
                    elif g == 10:
                        p_mix(j, HALFCO, EBLK)
            if MODE == "full":
                if b == BL - 1 and half == 1:
                    oeng.dma_start(
                        out_d[b, 128:256][:, 32:H, :], stage[:, 32:H, :])
                else:
                    oeng.dma_start(
                        out_d[b, half * 128 : half * 128 + 128], stage[:])
            elif MODE == "constout":
                oeng.dma_start(
                    out_d[b, half * 128 : half * 128 + 128], conststage[:])

    def body(include_next):
        for b in range(BL):
            j = (b + 2) % 4
            if not include_next and b >= 2:
                j = None
            if NOPREP:
                j = None
            conv(b, j)

    # prologue: router+kern for samples 0 and 1 (x0 in-DMA on SP, x1 on the
    # ACT queue so the two transfers overlap)
    prep_serial(0, nc.sync)
    prep_serial(1, eng2)
    if NOPREP:
        p_dma(2, nc.sync)
        p_gap(2); p_ph(2); p_relu(2); p_pl(2); p_soft(2); p_rb(2)
        p_mix(2, 0, EBLK)
        p_dma(3, eng2)
        p_gap(3); p_ph(3); p_relu(3); p_pl(3); p_soft(3); p_rb(3)
        p_mix(3, 0, EBLK)

    if UNROLL > 0:
        for _ in range(UNROLL):
            body(include_next=True)
    elif LOOP_REPS > 1:
        with tc.For_i(0, LOOP_REPS, 1, hint_engines=(mybir.EngineType.PE,)):
            body(include_next=True)
    else:
        body(include_next=False)

    if MODE != "full" and not TIMING:
        dummy_stage = stage_pool.tile([128, H, W], BF16, tag="stage")
        nc.vector.memset(dummy_stage[:], 0.0)
        nc.sync.dma_start(out_d[0, 0:128], dummy_stage[:])
    if TIMING:
        nc.sync.dma_start(sink_d[:], zf[0:1, 0:1])


def _dedup_ldweights(nc):
    """Delete an InstLdweights when the immediately-preceding surviving
    InstLdweights in the same block loaded the identical weights (same AP,
    perf mode, tile position) and the candidate carries no sync waits --
    the PE weight buffer still holds those weights, so the paired matmuls
    keep working and skip the reload."""
    fn = nc.m.functions[0]
    removed = 0
    for blk in fn.blocks:
        last_sig = None
        keep = []
        for i in blk.instructions:
            tn = type(i).__name__
            if tn == "InstLdweights":
                si = i.sync_info
                has_sync = si is not None and (len(si.on_wait) > 0
                                               or len(si.on_update) > 0)
                sig = (str(i.ins), str(i.perf_mode), str(i.tile_position),
                       str(i.is_transpose))
                if sig == last_sig and not has_sync:
                    removed += 1
                    continue  # drop it
                last_sig = sig
            elif tn != "InstMatmult":
                # any other PE-stream-relevant instruction invalidates
                # nothing (weights regs untouched by non-PE instructions),
                # but be conservative across control flow and barriers
                if tn in ("InstDrain", "InstEventSemaphore"):
                    pass  # sem ops don't touch the weight buffer
                else:
                    last_sig = None
            keep.append(i)
        if removed:
            blk.instructions.clear()
            for i in keep:
                blk.instructions.append(i)
    if removed:
        print(f"[kernel2] deduped {removed} InstLdweights")


def _strip_pe_sem_updates(nc):
    """Remove sem-inc updates from matmuls whose completion count nobody
    waits on, renumbering every wait/add/sub on that semaphore.

    The tile framework gives every PE instruction a then-inc on the PE
    event semaphore; consumers wait on absolute counts. Only ~14% of the
    counts are ever waited on (the last matmul of each PSUM tile and the
    router matmuls), but each inc costs a serialized EVT_SEM register
    write (~26ns) on the engine -- ~13us/iteration for 500+ matmuls.

    Model: a virtual count axis in block-layout order (which matches how
    the framework numbers waits: prologue counts, then body counts after
    the loop's sub-imm reset returns the value to the prologue total).
    """
    fn = nc.m.functions[0]
    sids = set()
    for blk in fn.blocks:
        for i in blk.instructions:
            if type(i).__name__ == "InstMatmult" and i.sync_info:
                for u in i.sync_info.on_update:
                    sids.add(u.id)
    if len(sids) != 1:
        return
    sid = sids.pop()

    mm_updates = []  # (inst, cum) for sem-inc-1 updates on sid
    waits = []       # SyncWait objects on sid
    addsubs = []     # (update_obj, cum_at_layout_pos)
    cum = 0
    ok = True
    for blk in fn.blocks:
        for i in blk.instructions:
            si = i.sync_info
            if si is None:
                continue
            for w in si.on_wait:
                if w.id == sid:
                    if w.wait_mode != "sem-ge-imm":
                        ok = False
                    waits.append(w)
            for u in si.on_update:
                if u.id != sid:
                    continue
                if u.update_mode == "sem-inc" and u.update_value == 1 \
                        and type(i).__name__ == "InstMatmult":
                    cum += 1
                    mm_updates.append((i, cum))
                elif u.update_mode in ("sem-add-imm", "sem-sub-imm"):
                    addsubs.append((u, cum))
                else:
                    ok = False
    if not ok or not mm_updates:
        return

    kept = sorted({v for w in waits for v in [w.wait_value]}
                  | {cum}
                  | {c + u.update_value for u, c in addsubs}
                  | {c for u, c in addsubs})
    import bisect

    def newv(v):
        return bisect.bisect_right(kept, v)

    kept_set = set(kept)
    removed = 0
    for inst, c in mm_updates:
        if c not in kept_set:
            si = inst.sync_info
            si.on_update = [u for u in si.on_update if u.id != sid]
            removed += 1
    for w in waits:
        w.wait_value = newv(w.wait_value)
    for u, c in addsubs:
        u.update_value = newv(c + u.update_value) - newv(c)
    log_note = f"stripped {removed}/{len(mm_updates)} PE sem updates"
    print(f"[kernel2] {log_note}")


_PROGRAM = None


def make_in_maps(inputs):
    import ml_dtypes

    x = np.asarray(inputs["x"], dtype=np.float32)
    convs = np.asarray(inputs["convs"], dtype=np.float32)
    w1 = np.asarray(inputs["w1"], dtype=np.float32)
    b1 = np.asarray(inputs["b1"], dtype=np.float32)
    w2 = np.asarray(inputs["w2"], dtype=np.float32)
    b2 = np.asarray(inputs["b2"], dtype=np.float32)

    xp = np.pad(x, ((0, 0), (0, 0), (1, 1), (1, 1))).astype(ml_dtypes.bfloat16)
    a = convs.transpose(2, 0, 1, 3, 4)          # [ci, e, co, kh, kw]
    a = a.reshape(CIN, E, 2, 128, KS, KS)       # co -> (half, co128)
    a = a.transpose(0, 1, 2, 4, 5, 3)           # [ci, e, half, kh, kw, co128]
    convs_r = np.ascontiguousarray(a).reshape(CIN, E * EBLK).astype(
        ml_dtypes.bfloat16)
    w1t = np.ascontiguousarray(w1.T) / float(HWN)
    b1c = np.ascontiguousarray(b1[:, None])
    w2t = np.ascontiguousarray(w2.T)
    g = np.ascontiguousarray(np.exp(b2 / TEMP)[None, :]).astype(np.float32)

    if TIMING:
        return [{"din": np.zeros((1, 1), np.float32)} for _ in range(NCORES)]
    return [
        {
            "x": np.ascontiguousarray(xp[c * BL : (c + 1) * BL]),
            "convs": convs_r,
            "w1t": w1t,
            "b1": b1c,
            "w2t": w2t,
            "g": g,
        }
        for c in range(NCORES)
    ]


def kernel(x, convs, w1, b1, w2, b2):
    global _PROGRAM, LAST_RESULTS
    if _PROGRAM is None:
        _PROGRAM = _build_program()
    nc = _PROGRAM

    in_maps = make_in_maps(dict(x=x, convs=convs, w1=w1, b1=b1, w2=w2, b2=b2))
    res = run_bass_kernel_spmd(nc, in_maps, core_ids=list(range(NCORES)),
                               trace=TRACE)
    LAST_RESULTS = res
    out = np.concatenate(
        [res.results[c]["out"].astype(np.float32) for c in range(NCORES)],
        axis=0)
    return out

